# revision 5
# baseline (speedup 1.0000x reference)
"""Trainium2 Bass kernel for a combined triplet + augmented-triplet (ATN) loss.

Strategy (8 NeuronCores, data-parallel over the batch dim):
  - Each core gets an 8192-row slice of anchor/positive/negative/embeddings.
  - Pass A streams all four tensors once (DMA-bound): fused row-dot stats for
    the triplet loss, one-hot label tiles, PE segment-sum matmuls for the
    class sums/counts, and PE transposes that leave E^T resident in SBUF.
  - AllReduce #1 combines class sums + counts across cores.
  - Small [16,x] compute derives normalized-centroid geometry (pair mask, cnt).
  - Pass B runs entirely from SBUF: G = E @ S^T via PE using the resident E^T,
    then relu terms are segment-summed per class with one more PE matmul.
  - AllReduce #2 combines the [16,17] segment sums + triplet partial sum;
    every core computes the identical final scalar.
"""

import numpy as np

import concourse.bacc as bacc
import concourse.bass as bass
import concourse.mybir as mybir
import concourse.tile as tile
from concourse.bass_utils import run_bass_kernel_spmd

F32 = mybir.dt.float32
I32 = mybir.dt.int32
OP = mybir.AluOpType
AF = mybir.ActivationFunctionType

NCORES = 8
N = 65536
D = 512
C = 16
PERCORE = N // NCORES          # 8192
TILES = PERCORE // 128         # 64
KCH = D // 128                 # 4 contraction chunks

TRIPLET_MARGIN = 0.2
ATN_ALPHA = 0.1
ATN_BETA = 1.05
EPS = 1e-8

_CACHE = {}


def ts(i, size):
    return bass.ts(i, size)


def build():
    nc = bacc.Bacc("TRN2", target_bir_lowering=False, debug=False,
                   num_devices=NCORES)

    a_d = nc.dram_tensor("anchor_s", [PERCORE, D], F32, kind="ExternalInput")
    p_d = nc.dram_tensor("positive_s", [PERCORE, D], F32, kind="ExternalInput")
    n_d = nc.dram_tensor("negative_s", [PERCORE, D], F32, kind="ExternalInput")
    e_d = nc.dram_tensor("emb_s", [PERCORE, D], F32, kind="ExternalInput")
    lab_d = nc.dram_tensor("labT", [128, TILES], F32, kind="ExternalInput")
    out_d = nc.dram_tensor("loss", [1, 1], F32, kind="ExternalOutput")

    with tile.TileContext(nc) as tc:
        with (
            tc.tile_pool(name="persist", bufs=1) as persist,
            tc.tile_pool(name="dram", bufs=1, space="DRAM") as dram,
        ):
            # ---- persistent SBUF state ----
            ET = persist.tile([128, KCH * PERCORE], F32)        # 128KB/part
            onehot_all = persist.tile([128, TILES * C], F32)    # 4KB/part
            ap_all = persist.tile([128, TILES], F32)
            an_all = persist.tile([128, TILES], F32)
            aa_all = persist.tile([128, TILES], F32)
            pp_all = persist.tile([128, TILES], F32)
            nn_all = persist.tile([128, TILES], F32)
            ee_all = persist.tile([128, TILES], F32)
            labsb = persist.tile([128, TILES], F32)
            cls_iota = persist.tile([128, C], F32)
            ident = persist.tile([128, 128], I32)
            identf = persist.tile([128, 128], F32)
            ones_col = persist.tile([128, 1], F32)
            bias_margin = persist.tile([128, 1], F32)
            bias_beta = persist.tile([128, 1], F32)
            bias_alpha = persist.tile([128, 1], F32)
            ones_row = persist.tile([1, 128], F32)
            Spack = persist.tile([16, D + 1], F32)
            ST_sb = persist.tile([128, KCH * C], F32)
            Sg = persist.tile([16, D + 1], F32)
            invs = persist.tile([16, 1], F32)
            invsRep = persist.tile([128, C], F32)
            presRep = persist.tile([16, C], F32)
            pairf = persist.tile([16, C], F32)
            cnt = persist.tile([16, 1], F32)
            inve_all = persist.tile([128, TILES], F32)
            neg_inve = persist.tile([128, TILES], F32)
            tripsb = persist.tile([1, 1], F32)
            Mpack = persist.tile([16, C + 2], F32)
            Mg = persist.tile([16, C + 2], F32)

            ar1_in = dram.tile([16, D + 1], F32)
            ar1_out = dram.tile([16, D + 1], F32)
            ar2_in = dram.tile([16, C + 2], F32)
            ar2_out = dram.tile([16, C + 2], F32)

            # ---- constants ----
            nc.sync.dma_start(labsb[:], lab_d.ap())
            nc.gpsimd.iota(cls_iota[:], [[1, C]], base=0, channel_multiplier=0,
                           allow_small_or_imprecise_dtypes=True)
            nc.gpsimd.iota(ident[:], [[1, 128]], base=0, channel_multiplier=-1)
            nc.vector.tensor_scalar(identf[:], ident[:], 0, None, OP.is_equal)
            nc.vector.memset(ones_col[:], 1.0)
            nc.vector.memset(bias_margin[:], TRIPLET_MARGIN)
            nc.vector.memset(bias_beta[:], ATN_BETA - 1.0)
            nc.vector.memset(bias_alpha[:], 1.0 - ATN_ALPHA)
            nc.vector.memset(ones_row[:], 1.0)

            ETv = ET[:].rearrange("q (k n) -> q k n", k=KCH)
            STv = ST_sb[:].rearrange("q (k c) -> q k c", k=KCH)

            # =============== PASS A ===============
            with (
                tc.tile_pool(name="stream", bufs=3) as stream,
                tc.tile_pool(name="scratch", bufs=6) as scratch,
                tc.tile_pool(name="psA", bufs=1, space="PSUM") as psA,
                tc.tile_pool(name="tp", bufs=2, space="PSUM") as tpp,
            ):
                S_ps = psA.tile([16, D], F32)
                cnt_ps = psA.tile([16, 1], F32)

                for t in range(TILES):
                    a_t = stream.tile([128, D], F32, tag="a")
                    p_t = stream.tile([128, D], F32, tag="p")
                    n_t = stream.tile([128, D], F32, tag="n")
                    e_t = stream.tile([128, D], F32, tag="e")
                    nc.sync.dma_start(a_t[:], a_d.ap()[ts(t, 128), :])
                    nc.sync.dma_start(p_t[:], p_d.ap()[ts(t, 128), :])
                    nc.sync.dma_start(n_t[:], n_d.ap()[ts(t, 128), :])
                    nc.sync.dma_start(e_t[:], e_d.ap()[ts(t, 128), :])

                    oh = onehot_all[:, ts(t, C)]
                    nc.vector.tensor_scalar(oh, cls_iota[:], labsb[:, ts(t, 1)],
                                            None, OP.is_equal)

                    first, last = (t == 0), (t == TILES - 1)
                    nc.tensor.matmul(S_ps[:], oh, e_t[:], start=first, stop=last)
                    nc.tensor.matmul(cnt_ps[:], oh, ones_col[:],
                                     start=first, stop=last)

                    tp = tpp.tile([128, D], F32)
                    for k in range(KCH):
                        nc.tensor.transpose(tp[:, ts(k, 128)],
                                            e_t[:, ts(k, 128)], identf[:])
                    nc.vector.tensor_copy(
                        ETv[:, :, ts(t, 128)],
                        tp[:].rearrange("q (k n) -> q k n", k=KCH))

                    s1 = scratch.tile([128, D], F32, tag="s")
                    nc.vector.scalar_tensor_tensor(
                        s1, a_t[:], 1.0, p_t[:], OP.mult, OP.mult,
                        accum_out=ap_all[:, ts(t, 1)])
                    s2 = scratch.tile([128, D], F32, tag="s")
                    nc.vector.scalar_tensor_tensor(
                        s2, a_t[:], 1.0, n_t[:], OP.mult, OP.mult,
                        accum_out=an_all[:, ts(t, 1)])
                    s3 = scratch.tile([128, D], F32, tag="s")
                    nc.scalar.activation(s3, a_t[:], AF.Square,
                                         accum_out=aa_all[:, ts(t, 1)])
                    s4 = scratch.tile([128, D], F32, tag="s")
                    nc.scalar.activation(s4, p_t[:], AF.Square,
                                         accum_out=pp_all[:, ts(t, 1)])
                    s5 = scratch.tile([128, D], F32, tag="s")
                    nc.scalar.activation(s5, e_t[:], AF.Square,
                                         accum_out=ee_all[:, ts(t, 1)])
                    s6 = scratch.tile([128, D], F32, tag="s")
                    nc.vector.scalar_tensor_tensor(
                        s6, n_t[:], 1.0, n_t[:], OP.mult, OP.mult,
                        accum_out=nn_all[:, ts(t, 1)])

                nc.vector.tensor_copy(Spack[:, :D], S_ps[:])
                nc.vector.tensor_copy(Spack[:, D:D + 1], cnt_ps[:])

            # =============== ALLREDUCE 1 ===============
            nc.gpsimd.dma_start(ar1_in[:], Spack[:])
            nc.gpsimd.collective_compute(
                "AllReduce", OP.add, replica_groups=[list(range(NCORES))],
                ins=[ar1_in.opt()], outs=[ar1_out.opt()])
            nc.gpsimd.dma_start(Sg[:], ar1_out[:])

            # =============== centroid geometry ([16,x] compute) ===========
            with (
                tc.tile_pool(name="psB", bufs=1, space="PSUM") as psB,
                tc.tile_pool(name="small", bufs=8) as small,
            ):
                # ST chunks: transpose Sg[:, :D] -> ST_sb [128, k, 16]
                for k in range(KCH):
                    stp = psB.tile([128, C], F32, tag="stp")
                    nc.tensor.transpose(stp[:], Sg[:16, ts(k, 128)],
                                        identf[:16, :16])
                    nc.vector.tensor_copy(STv[:, k, :], stp[:])

                ssm = psB.tile([16, C], F32, tag="ssm")
                for k in range(KCH):
                    nc.tensor.matmul(ssm[:], STv[:, k, :], STv[:, k, :],
                                     start=(k == 0), stop=(k == KCH - 1))

                norms2 = small.tile([16, 1], F32)
                sscr = small.tile([16, C], F32)
                nc.vector.scalar_tensor_tensor(
                    sscr, ssm[:], 1.0, identf[:16, :16], OP.mult, OP.mult,
                    accum_out=norms2[:])
                snorm = small.tile([16, 1], F32)
                nc.scalar.activation(snorm[:], norms2[:], AF.Sqrt)
                snorm2 = small.tile([16, 1], F32)
                nc.vector.tensor_scalar(snorm2[:], snorm[:], EPS, None, OP.max)
                nc.vector.reciprocal(invs[:], snorm2[:])

                # cos similarity matrix = ssm * invs_row * invs_col
                w1 = small.tile([16, C], F32)
                nc.vector.tensor_scalar(w1[:], ssm[:], invs[:], None, OP.mult)
                # replicate invs along partitions: invsRep = ones ⊗ invs^T
                ivT = psB.tile([1, C], F32, tag="ivT")
                nc.tensor.transpose(ivT[:], invs[:], identf[:16, :16])
                ivTs = small.tile([1, C], F32)
                nc.vector.tensor_copy(ivTs[:], ivT[:])
                ivR = psB.tile([128, C], F32, tag="ivR")
                nc.tensor.matmul(ivR[:], ones_row[:], ivTs[:])
                nc.vector.tensor_copy(invsRep[:], ivR[:])

                w2 = small.tile([16, C], F32)
                nc.vector.tensor_mul(w2[:], w1[:], invsRep[:16, :])

                # pair mask
                presC = small.tile([16, 1], F32)
                nc.vector.tensor_scalar(presC[:], Sg[:16, D:D + 1], 0.0, None,
                                        OP.is_gt)
                notEye = small.tile([16, C], F32)
                nc.vector.tensor_scalar(notEye[:], identf[:16, :16], 0.0, None,
                                        OP.is_equal)
                pf0 = small.tile([16, C], F32)
                nc.vector.tensor_scalar(pf0[:], w2[:], 1.0 - ATN_BETA, None,
                                        OP.is_ge)
                pf1 = small.tile([16, C], F32)
                nc.vector.scalar_tensor_tensor(pf1[:], pf0[:], presC[:],
                                               notEye[:], OP.mult, OP.mult)
                # presRep = ones ⊗ presC^T
                prT = psB.tile([1, C], F32, tag="ivT")
                nc.tensor.transpose(prT[:], presC[:], identf[:16, :16])
                prTs = small.tile([1, C], F32)
                nc.vector.tensor_copy(prTs[:], prT[:])
                prR = psB.tile([16, C], F32, tag="ivR")
                nc.tensor.matmul(prR[:], ones_row[:, :16], prTs[:])
                nc.vector.tensor_copy(presRep[:], prR[:])
                nc.vector.tensor_mul(pairf[:], pf1[:], presRep[:])
                nc.vector.tensor_reduce(cnt[:], pairf[:], mybir.AxisListType.X,
                                        OP.add)

                # per-row embedding inverse norms
                se = small.tile([128, TILES], F32, tag="big")
                nc.scalar.activation(se[:], ee_all[:], AF.Sqrt)
                se2 = small.tile([128, TILES], F32, tag="big")
                nc.vector.tensor_scalar(se2[:], se[:], EPS, None, OP.max)
                nc.vector.reciprocal(inve_all[:], se2[:])
                nc.vector.tensor_scalar(neg_inve[:], inve_all[:], -1.0, None,
                                        OP.mult)

                # ---- triplet partial sum (per-core) ----
                q1 = small.tile([128, TILES], F32, tag="big")
                nc.vector.tensor_mul(q1[:], aa_all[:], pp_all[:])
                q1s = small.tile([128, TILES], F32, tag="big")
                nc.scalar.activation(q1s[:], q1[:], AF.Sqrt)
                q1m = small.tile([128, TILES], F32, tag="big")
                nc.vector.tensor_scalar(q1m[:], q1s[:], EPS * EPS, None, OP.max)
                r1 = small.tile([128, TILES], F32, tag="big")
                nc.vector.reciprocal(r1[:], q1m[:])
                q2 = small.tile([128, TILES], F32, tag="big")
                nc.vector.tensor_mul(q2[:], aa_all[:], nn_all[:])
                q2s = small.tile([128, TILES], F32, tag="big")
                nc.scalar.activation(q2s[:], q2[:], AF.Sqrt)
                q2m = small.tile([128, TILES], F32, tag="big")
                nc.vector.tensor_scalar(q2m[:], q2s[:], EPS * EPS, None, OP.max)
                r2 = small.tile([128, TILES], F32, tag="big")
                nc.vector.reciprocal(r2[:], q2m[:])
                u1 = small.tile([128, TILES], F32, tag="big")
                nc.vector.tensor_mul(u1[:], ap_all[:], r1[:])
                u2 = small.tile([128, TILES], F32, tag="big")
                nc.vector.tensor_mul(u2[:], an_all[:], r2[:])
                v = small.tile([128, TILES], F32, tag="big")
                nc.vector.tensor_sub(v[:], u2[:], u1[:])
                vr = small.tile([128, TILES], F32, tag="big")
                tripcol = small.tile([128, 1], F32)
                nc.scalar.activation(vr[:], v[:], AF.Relu,
                                     bias=bias_margin[:], scale=1.0,
                                     accum_out=tripcol[:])
                trp = psB.tile([1, 1], F32, tag="trp")
                nc.tensor.matmul(trp[:], tripcol[:], ones_col[:])
                nc.vector.tensor_copy(tripsb[:], trp[:])

            # =============== PASS B (SBUF-resident) ===============
            with (
                tc.tile_pool(name="gp", bufs=2, space="PSUM") as gpp,
                tc.tile_pool(name="psM", bufs=1, space="PSUM") as psM,
                tc.tile_pool(name="rext", bufs=3) as rext_pool,
                tc.tile_pool(name="sm2", bufs=4) as sm2,
            ):
                M_ps = psM.tile([16, C + 1], F32)
                for t in range(TILES):
                    gp = gpp.tile([128, C], F32)
                    for k in range(KCH):
                        nc.tensor.matmul(gp[:], ETv[:, k, ts(t, 128)],
                                         STv[:, k, :],
                                         start=(k == 0), stop=(k == KCH - 1))
                    t1 = sm2.tile([128, C], F32, tag="t1")
                    nc.vector.tensor_mul(t1[:], gp[:], invsRep[:])
                    rext = rext_pool.tile([128, C + 1], F32)
                    nc.scalar.activation(rext[:, :C], t1[:], AF.Relu,
                                         bias=bias_beta[:],
                                         scale=inve_all[:, ts(t, 1)])
                    t1own = sm2.tile([128, 1], F32, tag="t1o")
                    sc16 = sm2.tile([128, C], F32, tag="sc16")
                    nc.vector.scalar_tensor_tensor(
                        sc16[:], t1[:], 1.0, onehot_all[:, ts(t, C)],
                        OP.mult, OP.mult, accum_out=t1own[:])
                    nc.scalar.activation(rext[:, C:C + 1], t1own[:], AF.Relu,
                                         bias=bias_alpha[:],
                                         scale=neg_inve[:, ts(t, 1)])
                    nc.tensor.matmul(M_ps[:], onehot_all[:, ts(t, C)], rext[:],
                                     start=(t == 0), stop=(t == TILES - 1))

                nc.vector.memset(Mpack[:], 0.0)
                nc.vector.tensor_copy(Mpack[:, :C + 1], M_ps[:])
                nc.vector.tensor_copy(Mpack[0:1, C + 1:C + 2], tripsb[:])

            # =============== ALLREDUCE 2 ===============
            nc.gpsimd.dma_start(ar2_in[:], Mpack[:])
            nc.gpsimd.collective_compute(
                "AllReduce", OP.add, replica_groups=[list(range(NCORES))],
                ins=[ar2_in.opt()], outs=[ar2_out.opt()])
            nc.gpsimd.dma_start(Mg[:], ar2_out[:])

            # =============== final scalar ===============
            with (
                tc.tile_pool(name="fin", bufs=4) as fin,
                tc.tile_pool(name="psF", bufs=1, space="PSUM") as psF,
            ):
                z1scr = fin.tile([16, C], F32)
                pack2 = fin.tile([16, 2], F32)
                nc.vector.scalar_tensor_tensor(
                    z1scr[:], Mg[:, :C], 1.0, pairf[:], OP.mult, OP.mult,
                    accum_out=pack2[:, 0:1])
                z2 = fin.tile([16, 1], F32)
                nc.vector.tensor_mul(z2[:], Mg[:, C:C + 1], cnt[:])
                nc.vector.tensor_add(pack2[:, 0:1], pack2[:, 0:1], z2[:])
                nc.vector.tensor_mul(pack2[:, 1:2], cnt[:], Sg[:16, D:D + 1])
                finp = psF.tile([2, 1], F32, tag="finp")
                nc.tensor.matmul(finp[:], pack2[:], ones_col[:16, :])
                finsb = fin.tile([2, 1], F32)
                nc.vector.tensor_copy(finsb[:], finp[:])
                finT = psF.tile([1, 2], F32, tag="finT")
                nc.tensor.transpose(finT[:], finsb[:], identf[:2, :2])
                finrow = fin.tile([1, 2], F32)
                nc.vector.tensor_copy(finrow[:], finT[:])
                # finrow = [[ZZ, NT]]
                m = fin.tile([1, 1], F32)
                nc.vector.tensor_scalar(m[:], finrow[:, 1:2], 1.0, None, OP.max)
                r = fin.tile([1, 1], F32)
                nc.vector.reciprocal(r[:], m[:])
                atn0 = fin.tile([1, 1], F32)
                nc.vector.tensor_mul(atn0[:], finrow[:, 0:1], r[:])
                gate = fin.tile([1, 1], F32)
                nc.vector.tensor_scalar(gate[:], finrow[:, 1:2], 0.0, None,
                                        OP.is_gt)
                atn = fin.tile([1, 1], F32)
                nc.vector.tensor_mul(atn[:], atn0[:], gate[:])
                losssb = fin.tile([1, 1], F32)
                nc.vector.scalar_tensor_tensor(
                    losssb[:], Mg[0:1, C + 1:C + 2], 1.0 / N, atn[:],
                    OP.mult, OP.add)
                nc.sync.dma_start(out_d.ap(), losssb[:])

    nc.compile()
    return nc


def _shard(inputs):
    anchor = np.ascontiguousarray(inputs["anchor"], dtype=np.float32)
    positive = np.ascontiguousarray(inputs["positive"], dtype=np.float32)
    negative = np.ascontiguousarray(inputs["negative"], dtype=np.float32)
    emb = np.ascontiguousarray(inputs["embeddings"], dtype=np.float32)
    labels = np.asarray(inputs["labels"], dtype=np.int32)
    in_maps = []
    for c in range(NCORES):
        sl = slice(c * PERCORE, (c + 1) * PERCORE)
        labT = np.ascontiguousarray(
            labels[sl].reshape(TILES, 128).T).astype(np.float32)
        in_maps.append({
            "anchor_s": anchor[sl],
            "positive_s": positive[sl],
            "negative_s": negative[sl],
            "emb_s": emb[sl],
            "labT": labT,
        })
    return in_maps


def run(inputs, trace=False):
    if "nc" not in _CACHE:
        _CACHE["nc"] = build()
    nc = _CACHE["nc"]
    in_maps = _shard(inputs)
    res = run_bass_kernel_spmd(nc, in_maps, core_ids=list(range(NCORES)),
                               trace=trace)
    loss = np.float32(res.results[0]["loss"][0, 0])
    return loss, res


def kernel(**inputs) -> np.ndarray:
    loss, _ = run(inputs, trace=False)
    return loss


# revision 6
# speedup vs baseline: 1.2522x; 1.2522x over previous
"""Trainium2 Bass kernel for a combined triplet + augmented-triplet (ATN) loss.

Strategy (8 NeuronCores, data-parallel over the batch dim):
  - Each core gets an 8192-row slice of anchor/positive/negative/embeddings.
  - Pass A streams all four tensors once (DMA-bound): fused row-dot stats for
    the triplet loss (f32), one-hot label tiles, bf16 PE segment-sum matmuls
    for the class sums/counts, and bf16 PE transposes that leave E^T resident
    in SBUF (no second HBM read).
  - AllReduce #1 combines class sums + counts across cores.
  - Small [16,x] compute derives normalized-centroid geometry (pair mask, cnt)
    in f32; the triplet partial sum + row norms overlap the allreduce.
  - Pass B runs entirely from SBUF: G = E @ S^T via bf16 PE matmuls on the
    resident E^T, relu terms segment-summed per class with one f32 PE matmul.
  - AllReduce #2 combines the [16,17] segment sums + triplet partial sum;
    every core computes the identical final scalar.
"""

import numpy as np

import concourse.bacc as bacc
import concourse.bass as bass
import concourse.mybir as mybir
import concourse.tile as tile
from concourse.bass_utils import run_bass_kernel_spmd

F32 = mybir.dt.float32
BF16 = mybir.dt.bfloat16
I32 = mybir.dt.int32
OP = mybir.AluOpType
AF = mybir.ActivationFunctionType

NCORES = 8
N = 65536
D = 512
C = 16
PERCORE = N // NCORES          # 8192
TILES = PERCORE // 128         # 64
ITERS = TILES // 2             # 2 row-tiles per DMA batch
KCH = D // 128                 # 4 contraction chunks

TRIPLET_MARGIN = 0.2
ATN_ALPHA = 0.1
ATN_BETA = 1.05
EPS = 1e-8

_CACHE = {}


def ts(i, size):
    return bass.ts(i, size)


def build():
    nc = bacc.Bacc("TRN2", target_bir_lowering=False, debug=False,
                   num_devices=NCORES)

    a_d = nc.dram_tensor("anchor_s", [PERCORE, D], F32, kind="ExternalInput")
    p_d = nc.dram_tensor("positive_s", [PERCORE, D], F32, kind="ExternalInput")
    n_d = nc.dram_tensor("negative_s", [PERCORE, D], F32, kind="ExternalInput")
    e_d = nc.dram_tensor("emb_s", [PERCORE, D], F32, kind="ExternalInput")
    lab_d = nc.dram_tensor("labT", [128, TILES], F32, kind="ExternalInput")
    out_d = nc.dram_tensor("loss", [1, 1], F32, kind="ExternalOutput")

    # [q, u, d] view of the [8192, 512] inputs: iteration m covers rows
    # 256m..256m+255 (partition q, sub-tile u in {0,1}).
    def rows2(dram, m):
        v = dram.ap().rearrange("(mm u q) d -> q mm u d", u=2, q=128)
        return v[:, m, :, :]

    with tile.TileContext(nc) as tc:
        with (
            tc.tile_pool(name="persist", bufs=1) as persist,
            tc.tile_pool(name="dram", bufs=1, space="DRAM") as dram,
        ):
            # ---- persistent SBUF state ----
            ET = persist.tile([128, KCH * PERCORE], BF16)       # 64KB/part
            onehot_all = persist.tile([128, TILES * C], F32)    # 4KB/part
            onehot_bf = persist.tile([128, TILES * C], BF16)    # 2KB/part
            ap_all = persist.tile([128, TILES], F32)
            an_all = persist.tile([128, TILES], F32)
            aa_all = persist.tile([128, TILES], F32)
            pp_all = persist.tile([128, TILES], F32)
            nn_all = persist.tile([128, TILES], F32)
            ee_all = persist.tile([128, TILES], F32)
            labsb = persist.tile([128, TILES], F32)
            cls_iota = persist.tile([128, C], F32)
            ident = persist.tile([128, 128], I32)
            identf = persist.tile([128, 128], F32)
            identb = persist.tile([128, 128], BF16)
            ones_col = persist.tile([128, 1], F32)
            ones_colb = persist.tile([128, 1], BF16)
            ones_row = persist.tile([1, 128], F32)
            bias_alpha = persist.tile([128, 1], F32)
            bias_margin = persist.tile([128, 1], F32)
            bias_beta = persist.tile([128, 1], F32)
            Spack = persist.tile([16, D + 1], F32)
            ST_sb = persist.tile([128, KCH * C], F32)
            ST_bf = persist.tile([128, KCH * C], BF16)
            Sg = persist.tile([16, D + 1], F32)
            invs = persist.tile([16, 1], F32)
            invsRep = persist.tile([128, C], F32)
            presRep = persist.tile([16, C], F32)
            pairf = persist.tile([16, C], F32)
            cnt = persist.tile([16, 1], F32)
            inve_all = persist.tile([128, TILES], F32)
            neg_inve = persist.tile([128, TILES], F32)
            tripsb = persist.tile([1, 1], F32)
            Mpack = persist.tile([16, C + 2], F32)
            Mg = persist.tile([16, C + 2], F32)

            ar1_in = dram.tile([16, D + 1], F32)
            ar1_out = dram.tile([16, D + 1], F32)
            ar2_in = dram.tile([16, C + 2], F32)
            ar2_out = dram.tile([16, C + 2], F32)

            # ---- constants ----
            nc.sync.dma_start(labsb[:], lab_d.ap())
            nc.gpsimd.iota(cls_iota[:], [[1, C]], base=0, channel_multiplier=0,
                           allow_small_or_imprecise_dtypes=True)
            nc.gpsimd.iota(ident[:], [[1, 128]], base=0, channel_multiplier=-1)
            nc.vector.tensor_scalar(identf[:], ident[:], 0, None, OP.is_equal)
            nc.vector.tensor_scalar(identb[:], ident[:], 0, None, OP.is_equal)
            nc.vector.memset(ones_col[:], 1.0)
            nc.vector.memset(ones_colb[:], 1.0)
            nc.vector.memset(ones_row[:], 1.0)
            nc.vector.memset(bias_alpha[:], 1.0 - ATN_ALPHA)
            nc.vector.memset(bias_margin[:], TRIPLET_MARGIN)
            nc.vector.memset(bias_beta[:], ATN_BETA - 1.0)

            ETv = ET[:].rearrange("q (k n) -> q k n", k=KCH)
            STv = ST_sb[:].rearrange("q (k c) -> q k c", k=KCH)
            STbv = ST_bf[:].rearrange("q (k c) -> q k c", k=KCH)

            # =============== PASS A ===============
            with (
                tc.tile_pool(name="stream", bufs=3) as stream,
                tc.tile_pool(name="scratch", bufs=6) as scratch,
                tc.tile_pool(name="psA", bufs=1, space="PSUM") as psA,
                tc.tile_pool(name="tp", bufs=2, space="PSUM") as tpp,
            ):
                S_ps = psA.tile([16, D], F32)
                cnt_ps = psA.tile([16, 1], F32)

                for m in range(ITERS):
                    a2 = stream.tile([128, 2, D], F32, tag="a")
                    p2 = stream.tile([128, 2, D], F32, tag="p")
                    n2 = stream.tile([128, 2, D], F32, tag="n")
                    e2 = stream.tile([128, 2, D], F32, tag="e")
                    nc.sync.dma_start(a2[:], rows2(a_d, m))
                    nc.sync.dma_start(p2[:], rows2(p_d, m))
                    nc.sync.dma_start(n2[:], rows2(n_d, m))
                    nc.sync.dma_start(e2[:], rows2(e_d, m))

                    eb2 = stream.tile([128, 2, D], BF16, tag="eb")
                    nc.vector.tensor_copy(eb2[:], e2[:])

                    # bf16 transposes of both sub-tiles into one PSUM bank
                    tp = tpp.tile([128, 2 * KCH * 128], BF16)
                    tpv = tp[:].rearrange("q (u k n) -> q u k n", u=2, k=KCH)
                    for u in range(2):
                        for k in range(KCH):
                            nc.tensor.transpose(tpv[:, u, k, :],
                                                eb2[:, u, ts(k, 128)],
                                                identb[:])
                    # one copy per iteration: (u,k,j) -> (k, u*128+j)
                    src = tp[:].rearrange("q (u k n) -> q k u n", u=2, k=KCH)
                    dst = ETv[:, :, ts(m, 256)].rearrange(
                        "q k (u n) -> q k u n", u=2)
                    nc.vector.tensor_copy(dst, src)

                    for u in range(2):
                        t = 2 * m + u
                        oh = onehot_all[:, ts(t, C)]
                        ohb = onehot_bf[:, ts(t, C)]
                        nc.vector.tensor_scalar(oh, cls_iota[:],
                                                labsb[:, ts(t, 1)],
                                                None, OP.is_equal)
                        nc.vector.tensor_scalar(ohb, cls_iota[:],
                                                labsb[:, ts(t, 1)],
                                                None, OP.is_equal)

                        first, last = (t == 0), (t == TILES - 1)
                        nc.tensor.matmul(S_ps[:], ohb, eb2[:, u, :],
                                         start=first, stop=last)
                        nc.tensor.matmul(cnt_ps[:], ohb, ones_colb[:],
                                         start=first, stop=last)

                        s1 = scratch.tile([128, D], F32, tag="s")
                        nc.vector.scalar_tensor_tensor(
                            s1, a2[:, u, :], 1.0, p2[:, u, :], OP.mult,
                            OP.mult, accum_out=ap_all[:, ts(t, 1)])
                        s2 = scratch.tile([128, D], F32, tag="s")
                        nc.vector.scalar_tensor_tensor(
                            s2, a2[:, u, :], 1.0, n2[:, u, :], OP.mult,
                            OP.mult, accum_out=an_all[:, ts(t, 1)])
                        s6 = scratch.tile([128, D], F32, tag="s")
                        nc.vector.scalar_tensor_tensor(
                            s6, n2[:, u, :], 1.0, n2[:, u, :], OP.mult,
                            OP.mult, accum_out=nn_all[:, ts(t, 1)])
                        s3 = scratch.tile([128, D], F32, tag="s")
                        nc.scalar.activation(s3, a2[:, u, :], AF.Square,
                                             accum_out=aa_all[:, ts(t, 1)])
                        s4 = scratch.tile([128, D], F32, tag="s")
                        nc.scalar.activation(s4, p2[:, u, :], AF.Square,
                                             accum_out=pp_all[:, ts(t, 1)])
                        s5 = scratch.tile([128, D], F32, tag="s")
                        nc.scalar.activation(s5, e2[:, u, :], AF.Square,
                                             accum_out=ee_all[:, ts(t, 1)])

                nc.vector.tensor_copy(Spack[:, :D], S_ps[:])
                nc.vector.tensor_copy(Spack[:, D:D + 1], cnt_ps[:])

            # =============== ALLREDUCE 1 ===============
            nc.gpsimd.dma_start(ar1_in[:], Spack[:])
            nc.gpsimd.collective_compute(
                "AllReduce", OP.add, replica_groups=[list(range(NCORES))],
                ins=[ar1_in.opt()], outs=[ar1_out.opt()])
            nc.gpsimd.dma_start(Sg[:], ar1_out[:])

            with (
                tc.tile_pool(name="psB", bufs=1, space="PSUM") as psB,
                tc.tile_pool(name="small", bufs=8) as small,
            ):
                # ---- triplet partial + row norms (pass-A stats only;
                # overlaps the allreduce latency) ----
                se = small.tile([128, TILES], F32, tag="big")
                nc.scalar.activation(se[:], ee_all[:], AF.Sqrt)
                se2 = small.tile([128, TILES], F32, tag="big")
                nc.vector.tensor_scalar(se2[:], se[:], EPS, None, OP.max)
                nc.vector.reciprocal(inve_all[:], se2[:])
                nc.vector.tensor_scalar(neg_inve[:], inve_all[:], -1.0, None,
                                        OP.mult)

                q1 = small.tile([128, TILES], F32, tag="big")
                nc.vector.tensor_mul(q1[:], aa_all[:], pp_all[:])
                q1s = small.tile([128, TILES], F32, tag="big")
                nc.scalar.activation(q1s[:], q1[:], AF.Sqrt)
                q1m = small.tile([128, TILES], F32, tag="big")
                nc.vector.tensor_scalar(q1m[:], q1s[:], EPS * EPS, None, OP.max)
                r1 = small.tile([128, TILES], F32, tag="big")
                nc.vector.reciprocal(r1[:], q1m[:])
                q2 = small.tile([128, TILES], F32, tag="big")
                nc.vector.tensor_mul(q2[:], aa_all[:], nn_all[:])
                q2s = small.tile([128, TILES], F32, tag="big")
                nc.scalar.activation(q2s[:], q2[:], AF.Sqrt)
                q2m = small.tile([128, TILES], F32, tag="big")
                nc.vector.tensor_scalar(q2m[:], q2s[:], EPS * EPS, None, OP.max)
                r2 = small.tile([128, TILES], F32, tag="big")
                nc.vector.reciprocal(r2[:], q2m[:])
                u1 = small.tile([128, TILES], F32, tag="big")
                nc.vector.tensor_mul(u1[:], ap_all[:], r1[:])
                u2 = small.tile([128, TILES], F32, tag="big")
                nc.vector.tensor_mul(u2[:], an_all[:], r2[:])
                v = small.tile([128, TILES], F32, tag="big")
                nc.vector.tensor_sub(v[:], u2[:], u1[:])
                vr = small.tile([128, TILES], F32, tag="big")
                tripcol = small.tile([128, 1], F32)
                nc.scalar.activation(vr[:], v[:], AF.Relu,
                                     bias=bias_margin[:], scale=1.0,
                                     accum_out=tripcol[:])
                trp = psB.tile([1, 1], F32, tag="trp")
                nc.tensor.matmul(trp[:], tripcol[:], ones_col[:])
                nc.vector.tensor_copy(tripsb[:], trp[:])

                # ---- centroid geometry (depends on allreduce 1) ----
                for k in range(KCH):
                    stp = psB.tile([128, C], F32, tag="stp")
                    nc.tensor.transpose(stp[:], Sg[:16, ts(k, 128)],
                                        identf[:16, :16])
                    nc.vector.tensor_copy(STv[:, k, :], stp[:])
                    nc.vector.tensor_copy(STbv[:, k, :], stp[:])

                ssm = psB.tile([16, C], F32, tag="ssm")
                for k in range(KCH):
                    nc.tensor.matmul(ssm[:], STv[:, k, :], STv[:, k, :],
                                     start=(k == 0), stop=(k == KCH - 1))

                norms2 = small.tile([16, 1], F32)
                sscr = small.tile([16, C], F32)
                nc.vector.scalar_tensor_tensor(
                    sscr, ssm[:], 1.0, identf[:16, :16], OP.mult, OP.mult,
                    accum_out=norms2[:])
                snorm = small.tile([16, 1], F32)
                nc.scalar.activation(snorm[:], norms2[:], AF.Sqrt)
                snorm2 = small.tile([16, 1], F32)
                nc.vector.tensor_scalar(snorm2[:], snorm[:], EPS, None, OP.max)
                nc.vector.reciprocal(invs[:], snorm2[:])

                w1 = small.tile([16, C], F32)
                nc.vector.tensor_scalar(w1[:], ssm[:], invs[:], None, OP.mult)
                ivT = psB.tile([1, C], F32, tag="ivT")
                nc.tensor.transpose(ivT[:], invs[:], identf[:16, :16])
                ivTs = small.tile([1, C], F32)
                nc.vector.tensor_copy(ivTs[:], ivT[:])
                ivR = psB.tile([128, C], F32, tag="ivR")
                nc.tensor.matmul(ivR[:], ones_row[:], ivTs[:])
                nc.vector.tensor_copy(invsRep[:], ivR[:])

                w2 = small.tile([16, C], F32)
                nc.vector.tensor_mul(w2[:], w1[:], invsRep[:16, :])

                presC = small.tile([16, 1], F32)
                nc.vector.tensor_scalar(presC[:], Sg[:16, D:D + 1], 0.0, None,
                                        OP.is_gt)
                notEye = small.tile([16, C], F32)
                nc.vector.tensor_scalar(notEye[:], identf[:16, :16], 0.0, None,
                                        OP.is_equal)
                pf0 = small.tile([16, C], F32)
                nc.vector.tensor_scalar(pf0[:], w2[:], 1.0 - ATN_BETA, None,
                                        OP.is_ge)
                pf1 = small.tile([16, C], F32)
                nc.vector.scalar_tensor_tensor(pf1[:], pf0[:], presC[:],
                                               notEye[:], OP.mult, OP.mult)
                prT = psB.tile([1, C], F32, tag="ivT")
                nc.tensor.transpose(prT[:], presC[:], identf[:16, :16])
                prTs = small.tile([1, C], F32)
                nc.vector.tensor_copy(prTs[:], prT[:])
                prR = psB.tile([16, C], F32, tag="ivR")
                nc.tensor.matmul(prR[:], ones_row[:, :16], prTs[:])
                nc.vector.tensor_copy(presRep[:], prR[:])
                nc.vector.tensor_mul(pairf[:], pf1[:], presRep[:])
                nc.vector.tensor_reduce(cnt[:], pairf[:], mybir.AxisListType.X,
                                        OP.add)

            # =============== PASS B (SBUF-resident, bf16 PE) ===============
            with (
                tc.tile_pool(name="gp", bufs=3, space="PSUM") as gpp,
                tc.tile_pool(name="psM", bufs=1, space="PSUM") as psM,
                tc.tile_pool(name="rext", bufs=4) as rext_pool,
                tc.tile_pool(name="sm2", bufs=4) as sm2,
            ):
                M_ps = psM.tile([16, C + 1], F32)
                for t in range(TILES):
                    gp = gpp.tile([128, C], F32)
                    for k in range(KCH):
                        nc.tensor.matmul(gp[:], ETv[:, k, ts(t, 128)],
                                         STbv[:, k, :],
                                         start=(k == 0), stop=(k == KCH - 1))
                    t1 = sm2.tile([128, C], F32, tag="t1")
                    nc.vector.tensor_mul(t1[:], gp[:], invsRep[:])
                    rext = rext_pool.tile([128, C + 1], F32)
                    nc.scalar.activation(rext[:, :C], t1[:], AF.Relu,
                                         bias=bias_beta[:],
                                         scale=inve_all[:, ts(t, 1)])
                    t1own = sm2.tile([128, 1], F32, tag="t1o")
                    sc16 = sm2.tile([128, C], F32, tag="sc16")
                    nc.vector.scalar_tensor_tensor(
                        sc16[:], t1[:], 1.0, onehot_all[:, ts(t, C)],
                        OP.mult, OP.mult, accum_out=t1own[:])
                    nc.scalar.activation(rext[:, C:C + 1], t1own[:], AF.Relu,
                                         bias=bias_alpha[:],
                                         scale=neg_inve[:, ts(t, 1)])
                    nc.tensor.matmul(M_ps[:], onehot_all[:, ts(t, C)], rext[:],
                                     start=(t == 0), stop=(t == TILES - 1))

                nc.vector.memset(Mpack[:], 0.0)
                nc.vector.tensor_copy(Mpack[:, :C + 1], M_ps[:])
                nc.vector.tensor_copy(Mpack[0:1, C + 1:C + 2], tripsb[:])

            # =============== ALLREDUCE 2 ===============
            nc.gpsimd.dma_start(ar2_in[:], Mpack[:])
            nc.gpsimd.collective_compute(
                "AllReduce", OP.add, replica_groups=[list(range(NCORES))],
                ins=[ar2_in.opt()], outs=[ar2_out.opt()])
            nc.gpsimd.dma_start(Mg[:], ar2_out[:])

            # =============== final scalar ===============
            with (
                tc.tile_pool(name="fin", bufs=4) as fin,
                tc.tile_pool(name="psF", bufs=1, space="PSUM") as psF,
            ):
                z1scr = fin.tile([16, C], F32)
                pack2 = fin.tile([16, 2], F32)
                nc.vector.scalar_tensor_tensor(
                    z1scr[:], Mg[:, :C], 1.0, pairf[:], OP.mult, OP.mult,
                    accum_out=pack2[:, 0:1])
                z2 = fin.tile([16, 1], F32)
                nc.vector.tensor_mul(z2[:], Mg[:, C:C + 1], cnt[:])
                nc.vector.tensor_add(pack2[:, 0:1], pack2[:, 0:1], z2[:])
                nc.vector.tensor_mul(pack2[:, 1:2], cnt[:], Sg[:16, D:D + 1])
                finp = psF.tile([2, 1], F32, tag="finp")
                nc.tensor.matmul(finp[:], pack2[:], ones_col[:16, :])
                finsb = fin.tile([2, 1], F32)
                nc.vector.tensor_copy(finsb[:], finp[:])
                finT = psF.tile([1, 2], F32, tag="finT")
                nc.tensor.transpose(finT[:], finsb[:], identf[:2, :2])
                finrow = fin.tile([1, 2], F32)
                nc.vector.tensor_copy(finrow[:], finT[:])
                m_ = fin.tile([1, 1], F32)
                nc.vector.tensor_scalar(m_[:], finrow[:, 1:2], 1.0, None,
                                        OP.max)
                r = fin.tile([1, 1], F32)
                nc.vector.reciprocal(r[:], m_[:])
                atn0 = fin.tile([1, 1], F32)
                nc.vector.tensor_mul(atn0[:], finrow[:, 0:1], r[:])
                gate = fin.tile([1, 1], F32)
                nc.vector.tensor_scalar(gate[:], finrow[:, 1:2], 0.0, None,
                                        OP.is_gt)
                atn = fin.tile([1, 1], F32)
                nc.vector.tensor_mul(atn[:], atn0[:], gate[:])
                losssb = fin.tile([1, 1], F32)
                nc.vector.scalar_tensor_tensor(
                    losssb[:], Mg[0:1, C + 1:C + 2], 1.0 / N, atn[:],
                    OP.mult, OP.add)
                nc.sync.dma_start(out_d.ap(), losssb[:])

    nc.compile()
    return nc


def _shard(inputs):
    anchor = np.ascontiguousarray(inputs["anchor"], dtype=np.float32)
    positive = np.ascontiguousarray(inputs["positive"], dtype=np.float32)
    negative = np.ascontiguousarray(inputs["negative"], dtype=np.float32)
    emb = np.ascontiguousarray(inputs["embeddings"], dtype=np.float32)
    labels = np.asarray(inputs["labels"], dtype=np.int32)
    in_maps = []
    for c in range(NCORES):
        sl = slice(c * PERCORE, (c + 1) * PERCORE)
        labT = np.ascontiguousarray(
            labels[sl].reshape(TILES, 128).T).astype(np.float32)
        in_maps.append({
            "anchor_s": anchor[sl],
            "positive_s": positive[sl],
            "negative_s": negative[sl],
            "emb_s": emb[sl],
            "labT": labT,
        })
    return in_maps


def run(inputs, trace=False):
    if "nc" not in _CACHE:
        _CACHE["nc"] = build()
    nc = _CACHE["nc"]
    in_maps = _shard(inputs)
    res = run_bass_kernel_spmd(nc, in_maps, core_ids=list(range(NCORES)),
                               trace=trace)
    loss = np.float32(res.results[0]["loss"][0, 0])
    return loss, res


def kernel(**inputs) -> np.ndarray:
    loss, _ = run(inputs, trace=False)
    return loss
